# revision 2
# baseline (speedup 1.0000x reference)
"""Trainium2 Bass kernel for nn_MixtureOfExpertsLayer (moe_routing) — v2.

Top-2 dispatch version: instead of computing all 4 experts densely for
every token (baseline), each expert only processes the tokens routed to
it.  Per core (1024 tokens, data-parallel across 8 cores):

  1. Router in fp32 (exact top-2 match vs the fp32 reference).
  2. On-device compaction: per expert, a value vector (token_id if
     selected else -1) is re-wrapped to the gpsimd 16-partition layout
     and compressed with `sparse_gather`, yielding the gather index
     list + dynamic count; gate values are compacted identically.
  3. `dma_gather(transpose=True)` pulls the selected tokens' rows from
     a bf16 token-major copy of x in DRAM directly into feature-major
     SBUF layout [128, H/128, C] (C = 640 capacity vs ~512 expected,
     observed per-core/expert max 553).
  4. Experts run in bf16 (full PE rate); the final down-projection is
     evicted token-major with the per-slot gate applied, and
     `dma_scatter_add` accumulates gate*expert(x) into the
     zero-initialized fp32 output rows.  Pad slots (idx -1) are skipped
     by the scatter.

The e1 (Math) chain of 3 consecutive linears and e2 (Code) wv@wo pair
are collapsed on the host into single matrices (weight-only algebra),
cutting per-token FLOPs ~9% on top of the ~2x from dispatch.
"""
import numpy as np

import concourse.bass as bass
import concourse.mybir as mybir
import concourse.tile as tile
from concourse import bacc
from concourse.alu_op_type import AluOpType
from concourse.bass_utils import run_bass_kernel_spmd

F32 = mybir.dt.float32
F32R = mybir.dt.float32r
BF16 = mybir.dt.bfloat16
I16 = mybir.dt.int16
U32 = mybir.dt.uint32
ACT = mybir.ActivationFunctionType
AX = mybir.AxisListType
OP = AluOpType

N_CORES = 8
B, S, H, I, E = 4, 2048, 1024, 4096, 4
TOK = (B * S) // N_CORES      # tokens per core
P = 128
KC = H // P                   # 8 feature chunks
KC2 = (2 * H) // P            # 16
TT = TOK // P                 # 8 token tiles
C = 640                       # per-expert capacity (observed max 553)
CT = C // P                   # 5 gathered token tiles
CW = C // 16                  # wrapped index columns
SLABS = ((0, 512), (512, 128))

BF16_W = {
    "sw_w1", "sw_w3", "sw_w2", "e1_w", "me_c1w", "me_c2w",
    "ce_syn_w", "att_w", "ce_f1w", "ce_f2w", "ce_gen_w",
    "ml_w1", "ml_w2", "xb", "me_c2b", "ce_gen_b", "ml_b2",
}

SHAPES = {
    "xt": [H, TOK], "xb": [TOK, H], "iota1": [P, TT], "wpos": [16, CW],
    "router_w": [H, E], "router_b": [E], "load_balancer": [E],
    "sw_w1": [H, I], "sw_w3": [H, I], "sw_w2": [I, H],
    "e1_w": [H, H], "e1_b": [H],
    "me_c1w": [H, 2 * H], "me_c1b": [2 * H],
    "me_c2w": [2 * H, H], "me_c2b": [H],
    "ce_syn_w": [H, H], "ce_syn_b": [H],
    "att_w": [H, H], "att_b": [H],
    "ce_n1g": [H], "ce_n1b": [H],
    "ce_f1w": [H, 2 * H], "ce_f1b": [2 * H],
    "ce_f2w": [2 * H, H], "ce_f2b": [H],
    "ce_n2g": [H], "ce_n2b": [H],
    "ce_gen_w": [H, H], "ce_gen_b": [H],
    "ml_w1": [H, I], "ml_b1": [I], "ml_w2": [I, H], "ml_b2": [H],
}


def build_moe2(debug=False):
    nc = bacc.Bacc("TRN2", target_bir_lowering=False, debug=debug,
                   num_swdge_queues=2)

    dt = {}
    for n, shp in SHAPES.items():
        d = BF16 if n in BF16_W else F32
        dt[n] = nc.dram_tensor(n, shp, d, kind="ExternalInput")
    # TOK real rows + 128 dummy rows absorbing pad-slot scatters
    out = nc.dram_tensor("out", [TOK + P, H], F32, kind="ExternalOutput")
    xb = dt["xb"]

    def wap(name):  # [rows, cols] -> [p, row_chunk, cols]
        return dt[name].ap().rearrange("(kc p) m -> p kc m", p=P)

    with tile.TileContext(nc) as tc:
        with (
            tc.tile_pool(name="const", bufs=1) as cpool,
            tc.tile_pool(name="rt", bufs=2) as rp,
            tc.tile_pool(name="gate", bufs=1) as gp,
            tc.tile_pool(name="disp", bufs=2) as dpool,
            tc.tile_pool(name="xg", bufs=2) as xgpool,
            tc.tile_pool(name="m10", bufs=2) as m10,
            tc.tile_pool(name="m20", bufs=2) as m20,
            tc.tile_pool(name="yout", bufs=2) as youtp,
            tc.tile_pool(name="wst", bufs=3) as wpool,
            tc.tile_pool(name="w2st", bufs=3) as w2pool,
            tc.tile_pool(name="lns", bufs=1) as lnsp,
            tc.tile_pool(name="sq", bufs=1) as sqp,
            tc.tile_pool(name="tmp", bufs=2) as tmpp,
            tc.tile_pool(name="ps", bufs=4, space=bass.MemorySpace.PSUM) as psp,
            tc.tile_pool(name="pss", bufs=2, space=bass.MemorySpace.PSUM) as pssp,
            tc.tile_pool(name="psb", bufs=2, space=bass.MemorySpace.PSUM) as psbp,
        ):
            # ---- constants ---------------------------------------------
            ones_cf = cpool.tile([P, 1], F32, tag="ones_cf")
            nc.vector.memset(ones_cf[:], 1.0)
            ones_cb = cpool.tile([P, 1], BF16, tag="ones_cb")
            nc.vector.tensor_copy(ones_cb[:], ones_cf[:])
            ones_cr = cpool.tile([P, 1], F32R, tag="ones_cr")
            nc.vector.tensor_copy(ones_cr[:], ones_cf[:])
            ones_rf = cpool.tile([1, P], F32, tag="ones_rf")
            nc.vector.memset(ones_rf[:], 1.0)
            ones_rb = cpool.tile([1, P], BF16, tag="ones_rb")
            nc.vector.tensor_copy(ones_rb[:], ones_rf[:])
            ones_rr = cpool.tile([1, P], F32R, tag="ones_rr")
            nc.vector.tensor_copy(ones_rr[:], ones_rf[:])
            epsH = cpool.tile([1, 1], F32, tag="epsH")
            nc.vector.memset(epsH[:], H * 1e-5)

            def const_bias(name, mc):
                t = cpool.tile([P, mc], F32, tag=name + "_cb")
                nc.sync.dma_start(t[:], dt[name].ap().rearrange(
                    "(mc p) -> p mc", p=P))
                return t

            def const_row_b(name):
                tb = cpool.tile([1, H], BF16, tag=name + "_rb")
                nc.sync.dma_start(tb[:], dt[name].ap().unsqueeze(0))
                return tb

            rb_f = cpool.tile([1, E], F32, tag="rb_f")
            nc.sync.dma_start(rb_f[:], dt["router_b"].ap().unsqueeze(0))
            lb_f = cpool.tile([1, E], F32, tag="lb_f")
            nc.sync.dma_start(lb_f[:], dt["load_balancer"].ap().unsqueeze(0))
            rblb = cpool.tile([1, E], F32, tag="rblb")
            nc.vector.tensor_tensor(rblb[:], rb_f[:], lb_f[:], OP.add)
            rw_sb = cpool.tile([P, KC, E], F32, tag="rw_sb")
            nc.sync.dma_start(rw_sb[:], wap("router_w"))
            iota1 = cpool.tile([P, TT], F32, tag="iota1")
            nc.sync.dma_start(iota1[:], dt["iota1"].ap())
            wpos = cpool.tile([16, CW], F32, tag="wpos")
            nc.sync.dma_start(wpos[:], dt["wpos"].ap())
            ones16 = cpool.tile([1, 16], F32, tag="ones16")
            nc.vector.memset(ones16[:], 1.0)

            # ---- zero-init output (incl. dummy rows) -------------------
            zt = cpool.tile([P, 1, 512], F32, tag="zt")
            nc.vector.memset(zt[:], 0.0)
            outv = out.ap().rearrange("(tt p) m -> p tt m", p=P)
            for t in range(TT + 1):
                for hh in range(2):
                    nc.sync.dma_start(
                        outv[:, t:t + 1, hh * 512:(hh + 1) * 512], zt[:])

            # ---- persistent gating state -------------------------------
            wgate = gp.tile([P, TT, E], F32, tag="wgate")
            sv = gp.tile([P, TT, E], F32, tag="sv")
            gv = gp.tile([P, TT, E], F32, tag="gv")
            ls = gp.tile([P, TT, E], F32, tag="ls")
            m1 = gp.tile([P, TT], F32, tag="m1")
            ind1 = gp.tile([P, TT, E], F32, tag="ind1")
            lm = gp.tile([P, TT, E], F32, tag="lm")
            m2 = gp.tile([P, TT], F32, tag="m2")
            dd = gp.tile([P, TT, E], F32, tag="dd")
            ed = gp.tile([P, TT, E], F32, tag="ed")
            ind2 = gp.tile([P, TT, E], F32, tag="ind2")
            wnum = gp.tile([P, TT, E], F32, tag="wnum")
            zz = gp.tile([P, TT], F32, tag="zz")
            rzz = gp.tile([P, TT], F32, tag="rzz")

            # ---- router (fp32, identical selection to reference).
            # PE part per 128-token tile; the elementwise chain is batched
            # per STEP across all 8 tiles so each engine streams
            # back-to-back instead of ping-ponging DVE<->ACT per tile.
            for t in range(TT):
                xf = rp.tile([P, KC, P], F32, tag="xf")
                nc.sync.dma_start(xf[:], wap("xt")[:, :, t * P:(t + 1) * P])
                lps = psp.tile([P, E], F32, tag="mm")
                for kc in range(KC):
                    nc.tensor.matmul(lps[:], xf[:, kc, :], rw_sb[:, kc, :],
                                     start=(kc == 0), stop=False)
                nc.tensor.matmul(lps[:], ones_rf[:], rblb[:],
                                 start=False, stop=True)
                nc.scalar.activation(ls[:, t, :], lps[:], ACT.Copy)
            for t in range(TT):
                nc.vector.tensor_reduce(m1[:, t:t + 1], ls[:, t, :], AX.X,
                                        OP.max)
            for t in range(TT):
                nc.vector.tensor_scalar(ind1[:, t, :], ls[:, t, :],
                                        m1[:, t:t + 1], -1e30,
                                        OP.is_ge, OP.mult)
            for t in range(TT):
                nc.vector.tensor_tensor(lm[:, t, :], ls[:, t, :],
                                        ind1[:, t, :], OP.add)
            for t in range(TT):
                nc.vector.tensor_reduce(m2[:, t:t + 1], lm[:, t, :], AX.X,
                                        OP.max)
            for t in range(TT):
                nc.vector.tensor_scalar(dd[:, t, :], ls[:, t, :],
                                        m1[:, t:t + 1], None, OP.subtract)
            for t in range(TT):
                nc.scalar.activation(ed[:, t, :], dd[:, t, :], ACT.Exp)
            for t in range(TT):
                nc.vector.tensor_scalar(ind2[:, t, :], ls[:, t, :],
                                        m2[:, t:t + 1], None, OP.is_ge)
            for t in range(TT):
                nc.vector.tensor_tensor(wnum[:, t, :], ed[:, t, :],
                                        ind2[:, t, :], OP.mult)
            for t in range(TT):
                nc.vector.tensor_reduce(zz[:, t:t + 1], wnum[:, t, :], AX.X,
                                        OP.add)
            nc.vector.reciprocal(rzz[:], zz[:])
            for t in range(TT):
                nc.vector.tensor_scalar(wgate[:, t, :], wnum[:, t, :],
                                        rzz[:, t:t + 1], None, OP.mult)
            for t in range(TT):
                # sel value: token_id if selected else -1
                nc.vector.tensor_scalar(sv[:, t, :], ind2[:, t, :],
                                        iota1[:, t:t + 1], -1.0,
                                        OP.mult, OP.add)
            for t in range(TT):
                # gate value: gate if selected else -1
                nc.vector.scalar_tensor_tensor(gv[:, t, :], ind2[:, t, :],
                                               -1.0, wgate[:, t, :],
                                               OP.add, OP.add)

            # ---- biases / rows -----------------------------------------
            e1b_t = const_bias("e1_b", KC)
            c1b_t = const_bias("me_c1b", KC2)
            synb_t = const_bias("ce_syn_b", KC)
            attb_t = const_bias("att_b", KC)
            f1b_t = const_bias("ce_f1b", KC2)
            f2b_t = const_bias("ce_f2b", KC)
            n1g_t = const_bias("ce_n1g", KC)
            n1b_t = const_bias("ce_n1b", KC)
            n2g_t = const_bias("ce_n2g", KC)
            n2b_t = const_bias("ce_n2b", KC)
            mlb1_t = const_bias("ml_b1", I // P)
            c2b_row = const_row_b("me_c2b")
            genb_row = const_row_b("ce_gen_b")
            mlb2_row = const_row_b("ml_b2")

            # ---- dispatch build (per expert) ---------------------------
            disp = {}

            def dispatch(e):
                svw = dpool.tile([16, TT * 8], F32, tag="svw")
                gvw = dpool.tile([16, TT * 8], F32, tag="gvw")
                for g in range(8):
                    nc.sync.dma_start(svw[:, g * TT:(g + 1) * TT],
                                      sv[g * 16:(g + 1) * 16, :, e])
                    nc.sync.dma_start(gvw[:, g * TT:(g + 1) * TT],
                                      gv[g * 16:(g + 1) * 16, :, e])
                svc = dpool.tile([16, CW], F32, tag="svc")
                nf = dpool.tile([1, 1], U32, tag="nf")
                nc.gpsimd.sparse_gather(svc[:], svw[:], num_found=nf[:])
                gvc = dpool.tile([16, CW], F32, tag="gvc")
                nf2 = dpool.tile([1, 1], U32, tag="nf2")
                nc.gpsimd.sparse_gather(gvc[:], gvw[:], num_found=nf2[:])
                # The HW ucode leaves garbage past num_found: mask the tail
                # against a broadcast count instead of trusting -1 pads.
                nf_f = dpool.tile([1, 1], F32, tag="nf_f")
                nc.vector.tensor_copy(nf_f[:], nf[:])
                nfp = psp.tile([16, 1], F32, tag="mm")
                nc.tensor.matmul(nfp[:], ones16[:], nf_f[:],
                                 start=True, stop=True)
                nfb = dpool.tile([16, 1], F32, tag="nfb")
                nc.vector.tensor_copy(nfb[:], nfp[:])
                # NaN-free cleanse of svc via int16 saturate round-trip
                svci = dpool.tile([16, CW], I16, tag="svci")
                nc.vector.tensor_copy(svci[:], svc[:])
                svcf = dpool.tile([16, CW], F32, tag="svcf")
                nc.vector.tensor_copy(svcf[:], svci[:])
                mask = dpool.tile([16, CW], F32, tag="mask")
                nc.vector.tensor_scalar(mask[:], wpos[:], nfb[:], None,
                                        OP.is_lt)
                idgf = dpool.tile([16, CW], F32, tag="idgf")
                nc.vector.tensor_tensor(idgf[:], svcf[:], mask[:], OP.mult)
                nd = dpool.tile([16, CW], F32, tag="nd")
                nc.vector.tensor_scalar(nd[:], mask[:], float(TOK),
                                        -float(TOK), OP.mult, OP.add)
                idsf = dpool.tile([16, CW], F32, tag="idsf")
                nc.vector.tensor_tensor(idsf[:], idgf[:], nd[:], OP.subtract)
                i16g = dpool.tile([16, CW], I16, tag="i16g")
                nc.vector.tensor_copy(i16g[:], idgf[:])
                i16s = dpool.tile([16, CW], I16, tag="i16s")
                nc.vector.tensor_copy(i16s[:], idsf[:])
                idxg = dpool.tile([128, CW], I16, tag="idxg")
                idxs = dpool.tile([128, CW], I16, tag="idxs")
                for g in range(8):
                    nc.sync.dma_start(idxg[g * 16:(g + 1) * 16, :], i16g[:])
                    nc.sync.dma_start(idxs[g * 16:(g + 1) * 16, :], i16s[:])
                gvm = dpool.tile([16, CW], F32, tag="gvm")
                nc.vector.tensor_tensor(gvm[:], gvc[:], mask[:], OP.mult)
                gate_tok = dpool.tile([128, CT], F32, tag="gt")
                gvc_v = gvm[:].rearrange("p (c f) -> p c f", f=8)
                for f8 in range(8):
                    nc.sync.dma_start(gate_tok[f8 * 16:(f8 + 1) * 16, :],
                                      gvc_v[:, :, f8])
                xg = xgpool.tile([P, KC, C], BF16, tag="xg")
                nc.gpsimd.dma_gather(xg[:], xb.ap(), idxg[:], C, C,
                                     elem_size=H, transpose=True,
                                     queue_num=1)
                disp[e] = (xg, gate_tok, idxs)

            def scatter(e, yout):
                _, _, idxs = disp[e]
                nc.gpsimd.dma_scatter_add(out.ap(), yout[:], idxs[:], C, C,
                                          elem_size=H, queue_num=0)

            # ---- compute helpers (bf16) --------------------------------
            def fm2(dst, w_name, n_mc, src, src_kc, act, bias_t=None,
                    bias_col0=0, w_col0=0, acc_dst=None, acc_bias=None):
                """dst[:, mc, :C] = act(W[:, cols].T @ src + b), bf16.

                Both token slabs accumulate in parallel PSUM banks so
                consecutive matmuls share the same stationary chunk.
                With acc_dst: dst[:, mc] = ps + acc_bias[mc] + acc_dst[:, mc]
                (f2 residual fusion) instead of an activation."""
                w_all = wap(w_name)
                for m0 in range(0, n_mc, 2):
                    wr = wpool.tile([P, src_kc, 256], BF16, tag="w")
                    nc.sync.dma_start(
                        wr[:], w_all[:, :src_kc,
                                     w_col0 + m0 * P:w_col0 + (m0 + 2) * P])
                    for ml in range(2):
                        mc = m0 + ml
                        psA = psp.tile([P, 512], F32, tag="mm")
                        psB = psp.tile([P, 512], F32, tag="mm")
                        pss = (psA, psB)
                        for kc in range(src_kc):
                            for si, (so, w) in enumerate(SLABS):
                                nc.tensor.matmul(
                                    pss[si][:, :w],
                                    wr[:, kc, ml * P:(ml + 1) * P],
                                    src[:, kc, so:so + w],
                                    start=(kc == 0), stop=(kc == src_kc - 1))
                        for si, (so, w) in enumerate(SLABS):
                            ps = pss[si]
                            if acc_dst is not None:
                                nc.vector.scalar_tensor_tensor(
                                    dst[:, mc, so:so + w], ps[:, :w],
                                    acc_bias[:, mc:mc + 1],
                                    acc_dst[:, mc, so:so + w],
                                    OP.add, OP.add)
                            elif bias_t is None:
                                nc.scalar.activation(dst[:, mc, so:so + w],
                                                     ps[:, :w], act)
                            else:
                                b_sl = bias_t[:, bias_col0 + mc:
                                              bias_col0 + mc + 1]
                                f = (ACT.Identity if act == ACT.Copy else act)
                                nc.scalar.activation(dst[:, mc, so:so + w],
                                                     ps[:, :w], f, bias=b_sl)

            def dproj(w_name, rb0, kcb, src, e, yout, init, bias_row=None):
                """yout[:, ct, :] (+)= gate * (src.T @ W[rb0 rows] + bias)."""
                _, gate_tok, _ = disp[e]
                w_all = wap(w_name)
                for hh in range(2):
                    nsb = (kcb + 7) // 8
                    wss = []
                    for sb in range(nsb):
                        kw = min(8, kcb - sb * 8)
                        ws = w2pool.tile([P, 8, 512], BF16, tag="w2")
                        nc.sync.dma_start(
                            ws[:, :kw, :],
                            w_all[:, rb0 + sb * 8:rb0 + sb * 8 + kw,
                                  hh * 512:(hh + 1) * 512])
                        wss.append((ws, kw))
                    for ct in range(CT):
                        ps = psp.tile([P, 512], F32, tag="mm")
                        for sb, (ws, kw) in enumerate(wss):
                            for kc in range(kw):
                                last = (sb == nsb - 1 and kc == kw - 1)
                                nc.tensor.matmul(
                                    ps[:], src[:, sb * 8 + kc,
                                               ct * P:(ct + 1) * P],
                                    ws[:, kc, :], start=(sb == 0 and kc == 0),
                                    stop=(last and bias_row is None))
                        if bias_row is not None:
                            nc.tensor.matmul(
                                ps[:], ones_rb[:],
                                bias_row[0:1, hh * 512:(hh + 1) * 512],
                                start=False, stop=True)
                        sl = yout[:, ct, hh * 512:(hh + 1) * 512]
                        g_sl = gate_tok[:, ct:ct + 1]
                        if init:
                            nc.vector.tensor_scalar(sl, ps[:], g_sl, None,
                                                    OP.mult)
                        else:
                            nc.vector.scalar_tensor_tensor(sl, ps[:], g_sl,
                                                           sl, OP.mult, OP.add)

            def layer_norm(dst, src, g_t, b_t):
                """dst(bf16) = LN(src)*g + b over features; src bf16."""
                for so, w in SLABS:
                    ssum = pssp.tile([1, 512], F32, tag="st")
                    for kc in range(KC):
                        nc.tensor.matmul(ssum[:, :w], ones_cb[:],
                                         src[:, kc, so:so + w],
                                         start=(kc == 0), stop=(kc == KC - 1))
                    ssq = pssp.tile([1, 512], F32, tag="st")
                    for kc in range(KC):
                        sq = sqp.tile([P, 1, 512], F32R, tag="sq")
                        nc.vector.tensor_tensor(
                            sq[:, :, :w], src[:, kc:kc + 1, so:so + w],
                            src[:, kc:kc + 1, so:so + w], OP.mult)
                        nc.tensor.matmul(
                            ssq[:, :w], ones_cr[:], sq[:, 0, :w],
                            start=(kc == 0), stop=(kc == KC - 1))
                    mu = lnsp.tile([1, 512], F32, tag="ln1")
                    nc.vector.tensor_scalar(mu[:, :w], ssum[:, :w], 1.0 / H,
                                            None, OP.mult)
                    # q = ssq - H*mu^2 = H*var; 1/sqrt(var+eps) =
                    # sqrt(H)/sqrt(q+H*eps) with sqrt(H) folded into g
                    # (host-scaled) — saves two row tiles.
                    q = lnsp.tile([1, 512], F32, tag="ln2")
                    nc.vector.tensor_tensor(q[:, :w], mu[:, :w], mu[:, :w],
                                            OP.mult)
                    nc.vector.scalar_tensor_tensor(q[:, :w], q[:, :w],
                                                   -float(H), ssq[:, :w],
                                                   OP.mult, OP.add)
                    sdev = lnsp.tile([1, 512], F32, tag="ln5")
                    nc.scalar.activation(sdev[:, :w], q[:, :w], ACT.Sqrt,
                                         bias=epsH[:])
                    rstd_f = lnsp.tile([1, 512], F32, tag="ln6")
                    nc.vector.reciprocal(rstd_f[:, :w], sdev[:, :w])
                    mub = psbp.tile([P, 512], F32, tag="bc")
                    nc.tensor.matmul(mub[:, :w], ones_rf[:], mu[:, :w],
                                     start=True, stop=True)
                    rsb = psbp.tile([P, 512], F32, tag="bc")
                    nc.tensor.matmul(rsb[:, :w], ones_rf[:], rstd_f[:, :w],
                                     start=True, stop=True)
                    for kc in range(KC):
                        t1 = tmpp.tile([P, 512], F32, tag="sa")
                        nc.vector.tensor_tensor(t1[:, :w],
                                                src[:, kc, so:so + w],
                                                mub[:, :w], OP.subtract)
                        nc.vector.tensor_tensor(t1[:, :w], t1[:, :w],
                                                rsb[:, :w], OP.mult)
                        nc.vector.tensor_scalar(dst[:, kc, so:so + w],
                                                t1[:, :w], g_t[:, kc:kc + 1],
                                                b_t[:, kc:kc + 1],
                                                OP.mult, OP.add)

            # ---- experts -----------------------------------------------
            w1_all, w3_all = wap("sw_w1"), wap("sw_w3")

            def expert0(yout, fill=None):
                xg = disp[0][0]
                for ih in range(2):
                    h1 = m20.tile([P, 16, C], BF16, tag="m20")
                    for m0 in range(0, 16, 2):
                        c0 = ih * 2048 + m0 * P
                        wa = wpool.tile([P, KC, 256], BF16, tag="w")
                        nc.sync.dma_start(wa[:], w1_all[:, :, c0:c0 + 256])
                        wb = wpool.tile([P, KC, 256], BF16, tag="w")
                        nc.sync.dma_start(wb[:], w3_all[:, :, c0:c0 + 256])
                        for ml in range(2):
                            mc = m0 + ml
                            psaA = psp.tile([P, 512], F32, tag="mm")
                            psaB = psp.tile([P, 512], F32, tag="mm")
                            psbA = psp.tile([P, 512], F32, tag="mm")
                            psbB = psp.tile([P, 512], F32, tag="mm")
                            pa, pb = (psaA, psaB), (psbA, psbB)
                            for kc in range(KC):
                                for si, (so, w) in enumerate(SLABS):
                                    nc.tensor.matmul(
                                        pa[si][:, :w],
                                        wa[:, kc, ml * P:(ml + 1) * P],
                                        xg[:, kc, so:so + w],
                                        start=(kc == 0), stop=(kc == KC - 1))
                            for kc in range(KC):
                                for si, (so, w) in enumerate(SLABS):
                                    nc.tensor.matmul(
                                        pb[si][:, :w],
                                        wb[:, kc, ml * P:(ml + 1) * P],
                                        xg[:, kc, so:so + w],
                                        start=(kc == 0), stop=(kc == KC - 1))
                            for si, (so, w) in enumerate(SLABS):
                                sa = tmpp.tile([P, 512], F32, tag="sa")
                                nc.scalar.activation(sa[:, :w], pa[si][:, :w],
                                                     ACT.Silu)
                                nc.vector.tensor_tensor(h1[:, mc, so:so + w],
                                                        pb[si][:, :w],
                                                        sa[:, :w], OP.mult)
                    # hide the next expert's first layer behind this dproj
                    # so its weight DMAs beat the scatter/gather burst
                    if ih == 1 and fill is not None:
                        fill()
                    dproj("sw_w2", ih * 16, 16, h1, 0, yout, init=(ih == 0))

            def e1_head():
                sym = m10.tile([P, KC, C], BF16, tag="m10")
                fm2(sym, "e1_w", KC, disp[1][0], KC, ACT.Copy, e1b_t)
                return sym

            def e2_head():
                syn = m10.tile([P, KC, C], BF16, tag="m10")
                fm2(syn, "ce_syn_w", KC, disp[2][0], KC, ACT.Copy, synb_t)
                return syn

            def expert1(yout, sym, fill=None):
                c1h = m20.tile([P, 16, C], BF16, tag="m20")
                fm2(c1h, "me_c1w", KC2, sym, KC, ACT.Gelu, c1b_t)
                if fill is not None:
                    fill()
                dproj("me_c2w", 0, 16, c1h, 1, yout, init=True,
                      bias_row=c2b_row)

            def e3_block(ih, yout):
                xg = disp[3][0]
                a = m20.tile([P, 16, C], BF16, tag="m20")
                fm2(a, "ml_w1", 16, xg, KC, ACT.Gelu, mlb1_t,
                    bias_col0=ih * 16, w_col0=ih * 2048)
                dproj("ml_w2", ih * 16, 16, a, 3, yout, init=(ih == 0),
                      bias_row=(mlb2_row if ih == 0 else None))

            def expert23(yout2, yout3, syn):
                """e2 with e3's blocks interleaved into the LN stalls."""
                syn2 = m10.tile([P, KC, C], BF16, tag="m10")
                fm2(syn2, "att_w", KC, syn, KC, ACT.Copy, attb_t)
                e3_block(0, yout3)
                h2 = m10.tile([P, KC, C], BF16, tag="m10")
                layer_norm(h2, syn2, n1g_t, n1b_t)
                ff1 = m20.tile([P, 16, C], BF16, tag="m20")
                fm2(ff1, "ce_f1w", KC2, h2, KC, ACT.Relu, f1b_t)
                # f2 in feature-major, fused with the h2 residual add
                ffb = m10.tile([P, KC, C], BF16, tag="m10")
                fm2(ffb, "ce_f2w", KC, ff1, KC2, ACT.Copy,
                    acc_dst=h2, acc_bias=f2b_t)
                e3_block(1, yout3)
                scatter(3, yout3)
                h2b = m10.tile([P, KC, C], BF16, tag="m10")
                layer_norm(h2b, ffb, n2g_t, n2b_t)
                dproj("ce_gen_w", 0, KC, h2b, 2, yout2, init=True,
                      bias_row=genb_row)

            dispatch(0)
            dispatch(1)
            box = {}
            yout0 = youtp.tile([P, CT, H], F32, tag="yout")
            expert0(yout0, fill=lambda: box.__setitem__("sym", e1_head()))
            scatter(0, yout0)
            dispatch(2)
            yout1 = youtp.tile([P, CT, H], F32, tag="yout")
            expert1(yout1, box["sym"],
                    fill=lambda: box.__setitem__("syn", e2_head()))
            scatter(1, yout1)
            dispatch(3)
            yout2 = youtp.tile([P, CT, H], F32, tag="yout")
            yout3 = youtp.tile([P, CT, H], F32, tag="yout")
            expert23(yout2, yout3, box["syn"])
            scatter(2, yout2)

    nc.compile()
    return nc


_PROGRAM = None


def _get_program():
    global _PROGRAM
    if _PROGRAM is None:
        _PROGRAM = build_moe2()
    return _PROGRAM


def run_cores(nc, in_maps, trace=False, trace_cores=None):
    if trace:
        _install_ntff_shim()
    return run_bass_kernel_spmd(nc, in_maps, core_ids=list(range(len(in_maps))),
                                trace=trace, trace_cores=trace_cores)


def make_in_maps(inputs):
    import ml_dtypes
    bf16 = ml_dtypes.bfloat16
    f32 = np.float32
    g = {k: np.asarray(v, f32) for k, v in inputs.items()}

    # host-side weight algebra (collapse e1 linear chain, e2 wv@wo)
    e1_w = (g["me_eq_w"] @ g["me_wv"]) @ g["me_wo"]
    e1_b = (g["me_eq_b"] @ g["me_wv"] + g["me_bv"]) @ g["me_wo"] + g["me_bo"]
    att_w = np.eye(H, dtype=f32) + g["ce_wv"] @ g["ce_wo"]
    att_b = g["ce_bv"] @ g["ce_wo"] + g["ce_bo"]

    base = {
        "router_w": g["router_w"], "router_b": g["router_b"],
        "load_balancer": g["load_balancer"],
        "sw_w1": g["sw_w1"].astype(bf16), "sw_w3": g["sw_w3"].astype(bf16),
        "sw_w2": g["sw_w2"].astype(bf16),
        "e1_w": e1_w.astype(bf16), "e1_b": e1_b,
        "me_c1w": g["me_c1w"].astype(bf16), "me_c1b": g["me_c1b"],
        "me_c2w": g["me_c2w"].astype(bf16), "me_c2b": g["me_c2b"].astype(bf16),
        "ce_syn_w": g["ce_syn_w"].astype(bf16), "ce_syn_b": g["ce_syn_b"],
        "att_w": att_w.astype(bf16), "att_b": att_b,
        "ce_n1g": g["ce_n1g"] * np.sqrt(np.float32(H)),
        "ce_n1b": g["ce_n1b"],
        "ce_f1w": g["ce_f1w"].astype(bf16), "ce_f1b": g["ce_f1b"],
        "ce_f2w": g["ce_f2w"].astype(bf16), "ce_f2b": g["ce_f2b"],
        "ce_n2g": g["ce_n2g"] * np.sqrt(np.float32(H)),
        "ce_n2b": g["ce_n2b"],
        "ce_gen_w": g["ce_gen_w"].astype(bf16),
        "ce_gen_b": g["ce_gen_b"].astype(bf16),
        "ml_w1": g["ml_w1"].astype(bf16), "ml_b1": g["ml_b1"],
        "ml_w2": g["ml_w2"].astype(bf16), "ml_b2": g["ml_b2"].astype(bf16),
    }
    base = {k: np.ascontiguousarray(v) for k, v in base.items()}
    iota = np.ascontiguousarray(
        np.arange(TOK, dtype=f32).reshape(TT, P).T + 1.0)
    wpos = np.ascontiguousarray(
        np.arange(C, dtype=f32).reshape(CW, 16).T)

    x = g["x"].reshape(-1, H)
    in_maps = []
    for c in range(N_CORES):
        xs = x[c * TOK:(c + 1) * TOK]
        in_maps.append({
            **base,
            "xt": np.ascontiguousarray(xs.T),
            "xb": np.ascontiguousarray(xs.astype(bf16)),
            "iota1": iota,
            "wpos": wpos,
        })
    return in_maps


def kernel(**inputs):
    nc = _get_program()
    res = run_cores(nc, make_in_maps(inputs))
    outs = [res.results[c]["out"][:TOK] for c in range(N_CORES)]
    x = np.asarray(inputs["x"])
    return np.concatenate(outs, 0).reshape(x.shape).astype(np.float32)


# ---- NTFF profiling shim (axon) — used by test.py only ----------------
def _install_ntff_shim():
    import contextlib
    import ctypes
    import sys
    import types

    if "antenv.axon_hooks" in sys.modules:
        return
    lib = ctypes.CDLL("/opt/axon/libaxon_pjrt.so")
    if not hasattr(lib, "axon_start_nrt_profile"):
        return
    lib.axon_start_nrt_profile.argtypes = [ctypes.POINTER(ctypes.c_int64),
                                           ctypes.c_size_t]
    lib.axon_start_nrt_profile.restype = ctypes.c_int64
    lib.axon_stop_nrt_profile.argtypes = [ctypes.c_char_p]
    lib.axon_stop_nrt_profile.restype = ctypes.c_int64

    @contextlib.contextmanager
    def _hook(output_dir, device_ids):
        import jax
        jax.devices()
        if device_ids:
            ids = (ctypes.c_int64 * len(device_ids))(*device_ids)
            rc = lib.axon_start_nrt_profile(ids, len(device_ids))
        else:
            rc = lib.axon_start_nrt_profile(None, 0)
        if rc != 0:
            raise RuntimeError(f"axon_start_nrt_profile rc={rc}")
        try:
            yield
        finally:
            n = lib.axon_stop_nrt_profile(str(output_dir).encode())
            print(f"profile: {n} file(s) written to {output_dir}",
                  file=sys.stderr)

    import antenv
    mod = types.ModuleType("antenv.axon_hooks")
    mod.get_axon_ntff_profile_hook = lambda: _hook
    mod.set_axon_ntff_profile_hook = lambda hk: None
    sys.modules["antenv.axon_hooks"] = mod
    antenv.axon_hooks = mod
